# revision 10
# baseline (speedup 1.0000x reference)
"""Trainium2 Bass kernel for nn_CCMoE (chart-cover MoE policy head).

Strategy: pure data-parallel over 8 NeuronCores (batch sharding). Each core
processes B/8 = 16384 samples in feature-major layout ([feature, batch] tiles,
batch tiled by T=512 columns). All contractions run on the tensor engine,
including partition-dim reductions (ones / replication / group-sum matrices as
stationary operands). Whitening, the locality prior, and all biases are folded
into matmul weights plus activation bias ports; the squared-distance term
||zw||^2 is accumulated directly into the [16,T] distance tile via
column-replicated weights. The masked softmax is computed unnormalized (logits
are bounded, exp-safe); normalization via fast-Newton reciprocal + K=1
broadcast matmul.

Self-contained: shapes/sharding hardcoded for the fixed problem size.
"""
import numpy as np
import ml_dtypes
from contextlib import ExitStack

import concourse.bass as bass
import concourse.bacc as bacc
import concourse.tile as tile
from concourse import mybir
from concourse import bass_utils

# problem dims (hardcoded per contract)
B, OBS, ACTD, NE, HID, GHID = 131072, 17, 6, 16, 256, 128
NCORES = 8
BC = B // NCORES          # 16384 samples per core
T = 512                   # batch-column tile (one fp32 PSUM bank)
NT = BC // T              # 32 tiles per core
R = 24.0
BETA = 1.0
S = BETA / (R * R)
LSMIN, LSMAX = -20.0, 2.0

F32 = mybir.dt.float32
F32R = mybir.dt.float32r
BF16 = mybir.dt.bfloat16
AT = mybir.ActivationFunctionType
OP = mybir.AluOpType

MM_DTYPE = "bf16"         # matmul input dtype: "bf16" | "f32r" | "f32"


def _mmdt():
    return {"bf16": BF16, "f32r": F32R, "f32": F32}[MM_DTYPE]


def to_mm(x):
    """Convert host fp32 array to the matmul input dtype's storage."""
    x = np.ascontiguousarray(x, np.float32)
    if MM_DTYPE == "bf16":
        return np.ascontiguousarray(x.astype(ml_dtypes.bfloat16))
    if MM_DTYPE == "f32r":
        xi = x.view(np.uint32)
        yi = ((xi.astype(np.uint64) + 0x800) & 0xFFFFF000).astype(np.uint32)
        return np.ascontiguousarray(yi.view(np.float32))
    return x


def build_nc():
    MMDT = _mmdt()
    nc = bacc.Bacc("TRN2", target_bir_lowering=False, debug=False)

    dram_in = lambda name, shape: nc.dram_tensor(name, shape, MMDT, kind="ExternalInput").ap()
    obs_t = dram_in("obs_t", [OBS, BC])
    w1 = dram_in("w1", [OBS, HID])
    w2 = dram_in("w2", [HID, HID])
    wg1 = dram_in("wg1", [HID, GHID])
    wg2 = dram_in("wg2", [GHID, NE])
    wv1 = dram_in("wv1", [HID, HID])
    wv2 = dram_in("wv2", [HID, HID])
    wv3 = dram_in("wv3", [HID, 1])
    fcen = dram_in("fcen", [HID, NE])        # -2S*inv*(c + inv*mean), per column i
    brep = dram_in("brep", [HID, NE])        # S*inv^2 replicated across columns
    muwt = dram_in("muwt", [HID, NE * ACTD])
    mub16 = dram_in("mub16", [NE, ACTD])
    ls16 = dram_in("ls16", [NE, ACTD])
    rep16 = dram_in("rep16", [NE, NE * ACTD])
    grp96 = dram_in("grp96", [NE * ACTD, ACTD])
    ones16_d = dram_in("ones16_d", [NE, 1])
    ones116_d = dram_in("ones116_d", [1, NE])
    bias128 = nc.dram_tensor("bias128", [128, 9], F32, kind="ExternalInput").ap()
    bias16 = nc.dram_tensor("bias16", [NE, 2], F32, kind="ExternalInput").ap()
    bv3s = nc.dram_tensor("bv3s", [1, 1], F32, kind="ExternalInput").ap()
    out13 = nc.dram_tensor("out13", [13, BC], F32, kind="ExternalOutput").ap()

    with tile.TileContext(nc) as tc:
        with ExitStack() as ctx:
            cp = ctx.enter_context(tc.tile_pool(name="consts", bufs=1))
            io = ctx.enter_context(tc.tile_pool(name="io", bufs=4))
            wk = ctx.enter_context(tc.tile_pool(name="work", bufs=3))
            ps_enc = ctx.enter_context(tc.tile_pool(name="ps_enc", bufs=2, space="PSUM"))
            ps_val = ctx.enter_context(tc.tile_pool(name="ps_val", bufs=1, space="PSUM"))
            ps_sm = ctx.enter_context(tc.tile_pool(name="ps_sm", bufs=2, space="PSUM"))

            # ---- constants into SBUF ----
            def cload(name, src, shape, dt=MMDT):
                t = cp.tile(shape, dt, tag=name)
                nc.sync.dma_start(t[:], src)
                return t

            w1s = cload("w1s", w1[:], [OBS, HID])
            w2c = [cload(f"w2_{k}", w2[k * 128:(k + 1) * 128, :], [128, HID]) for k in range(2)]
            wg1c = [cload(f"wg1_{k}", wg1[k * 128:(k + 1) * 128, :], [128, GHID]) for k in range(2)]
            wg2s = cload("wg2s", wg2[:], [GHID, NE])
            wv1c = [cload(f"wv1_{k}", wv1[k * 128:(k + 1) * 128, :], [128, HID]) for k in range(2)]
            wv2c = [cload(f"wv2_{k}", wv2[k * 128:(k + 1) * 128, :], [128, HID]) for k in range(2)]
            wv3c = [cload(f"wv3_{k}", wv3[k * 128:(k + 1) * 128, :], [128, 1]) for k in range(2)]
            fcenc = [cload(f"fcen_{k}", fcen[k * 128:(k + 1) * 128, :], [128, NE]) for k in range(2)]
            brepc = [cload(f"brep_{k}", brep[k * 128:(k + 1) * 128, :], [128, NE]) for k in range(2)]
            muwtc = [cload(f"muwt_{k}", muwt[k * 128:(k + 1) * 128, :], [128, NE * ACTD]) for k in range(2)]
            mubs = cload("mubs", mub16[:], [NE, ACTD])
            lss = cload("lss", ls16[:], [NE, ACTD])
            reps = cload("reps", rep16[:], [NE, NE * ACTD])
            grps = cload("grps", grp96[:], [NE * ACTD, ACTD])
            ones16 = cload("ones16", ones16_d[:], [NE, 1])
            ones116 = cload("ones116", ones116_d[:], [1, NE])
            b128 = cload("b128", bias128[:], [128, 9], dt=F32)
            b16 = cload("b16", bias16[:], [NE, 2], dt=F32)
            bv3t = cload("bv3t", bv3s[:], [1, 1], dt=F32)

            bg2a = b16[:, 0:1]
            cbias = b16[:, 1:2]

            # bias slots in b128 (columns): 0,1=b1  2,3=b2  4=bg1  5,6=bv1  7,8=bv2
            bias_zero = build_nc.bias_zero  # host-detected zero-bias flags

            def mm(out, lhsT, rhs, start=True, stop=True):
                nc.tensor.matmul(out, lhsT, rhs, start=start, stop=stop)

            def tanh2(out_tile, psum, cols_ab, zero):
                """tanh over a [128, 2T] psum pair; merged when bias is zero."""
                ca, cb = cols_ab
                if zero:
                    nc.scalar.activation(out_tile[:], psum[:], AT.Tanh, bias=0.0)
                else:
                    nc.scalar.activation(out_tile[:, 0:T], psum[:, 0:T], AT.Tanh, bias=b128[:, ca:ca + 1])
                    nc.scalar.activation(out_tile[:, T:2 * T], psum[:, T:2 * T], AT.Tanh, bias=b128[:, cb:cb + 1])

            # ---- main loop over batch-column tiles ----
            for i in range(NT):
                cols = slice(i * T, (i + 1) * T)

                obs_s = io.tile([OBS, T], MMDT, tag="obs_s")
                nc.sync.dma_start(obs_s[:], obs_t[:, cols])

                # encoder layer 1: h1 = tanh(W1.T @ obs + b1)  [256, T]
                ph1 = ps_enc.tile([128, 2 * T], F32, tag="penc")
                mm(ph1[:, 0:T], w1s[:, 0:128], obs_s[:])
                mm(ph1[:, T:2 * T], w1s[:, 128:256], obs_s[:])
                h1 = wk.tile([128, 2 * T], MMDT, tag="h1")
                tanh2(h1, ph1, (0, 1), bias_zero["b1"])

                # encoder layer 2: feat = tanh(W2.T @ h1 + b2)  [256, T]
                pf = ps_enc.tile([128, 2 * T], F32, tag="penc")
                for m in range(2):
                    mm(pf[:, m * T:(m + 1) * T], w2c[0][:, m * 128:(m + 1) * 128], h1[:, 0:T], start=True, stop=False)
                    mm(pf[:, m * T:(m + 1) * T], w2c[1][:, m * 128:(m + 1) * 128], h1[:, T:2 * T], start=False, stop=True)
                feat = wk.tile([128, 2 * T], MMDT, tag="feat")
                tanh2(feat, pf, (2, 3), bias_zero["b2"])

                # gate hidden: gh = tanh(Wg1.T @ feat + bg1)  [128, T]
                pgh = ps_sm.tile([128, T], F32, tag="psm")
                mm(pgh[:], wg1c[0][:], feat[:, 0:T], start=True, stop=False)
                mm(pgh[:], wg1c[1][:], feat[:, T:2 * T], start=False, stop=True)
                gh = wk.tile([128, T], MMDT, tag="gh")
                if bias_zero["bg1"]:
                    nc.scalar.activation(gh[:], pgh[:], AT.Tanh, bias=0.0)
                else:
                    nc.scalar.activation(gh[:], pgh[:], AT.Tanh, bias=b128[:, 4:5])

                # gate logits (pre-prior): glog = Wg2.T @ gh   [16, T]
                pglog = ps_sm.tile([NE, T], F32, tag="psm")
                mm(pglog[:], wg2s[:], gh[:])

                # feat^2 for ||zw||^2
                f2 = wk.tile([128, 2 * T], MMDT, tag="f2")
                fview = feat[:].bitcast(F32) if MM_DTYPE == "f32r" else feat[:]
                nc.vector.tensor_tensor(f2[:], fview, fview, OP.mult)

                # d2s = relu(S*d2) = relu(fcen.T@feat + brep.T@feat^2 + cbias)  [16, T]
                pd2 = ps_sm.tile([NE, T], F32, tag="psm")
                mm(pd2[:], fcenc[0][:], feat[:, 0:T], start=True, stop=False)
                mm(pd2[:], fcenc[1][:], feat[:, T:2 * T], start=False, stop=False)
                mm(pd2[:], brepc[0][:], f2[:, 0:T], start=False, stop=False)
                mm(pd2[:], brepc[1][:], f2[:, T:2 * T], start=False, stop=True)
                d2s = wk.tile([NE, T], F32, tag="d2s")
                nc.vector.tensor_scalar(d2s[:], pd2[:], cbias, 0.0, OP.add, OP.max)

                # masked, unnormalized softmax: e = (d2s<=BETA) * exp(glog - d2s + bg2)
                tl = wk.tile([NE, T], F32, tag="tl")
                nc.vector.scalar_tensor_tensor(tl[:], d2s[:], -1.0, pglog[:], OP.mult, OP.add)
                eraw = wk.tile([NE, T], F32, tag="eraw")
                nc.scalar.activation(eraw[:], tl[:], AT.Exp, bias=bg2a)
                e = wk.tile([NE, T], MMDT, tag="e")
                nc.vector.scalar_tensor_tensor(e[:], d2s[:], BETA, eraw[:], OP.is_le, OP.mult)

                # Z = sum_m e;  rz = 1/Z;  en = e * bcast(rz)
                pZ = ps_sm.tile([1, T], F32, tag="psm")
                mm(pZ[:], ones16[:], e[:])
                rz32 = wk.tile([1, T], F32, tag="rz32")
                nc.vector.reciprocal_approx_fast(rz32[:], pZ[:])
                rz = wk.tile([1, T], MMDT, tag="rz")
                nc.vector.tensor_copy(rz[:], rz32[:])
                prb = ps_sm.tile([NE, T], F32, tag="psm")
                mm(prb[:], ones116[:], rz[:])
                en = wk.tile([NE, T], MMDT, tag="en")
                eview = e[:].bitcast(F32) if MM_DTYPE == "f32r" else e[:]
                nc.vector.tensor_tensor(en[:], eview, prb[:], OP.mult)

                # experts: mus = muwt.T @ feat  [96, T]; prod = mus * rep(en)
                pmus = ps_sm.tile([NE * ACTD, T], F32, tag="psm")
                mm(pmus[:], muwtc[0][:], feat[:, 0:T], start=True, stop=False)
                mm(pmus[:], muwtc[1][:], feat[:, T:2 * T], start=False, stop=True)
                mus = wk.tile([NE * ACTD, T], F32, tag="mus")
                nc.vector.tensor_copy(mus[:], pmus[:])
                prep = ps_sm.tile([NE * ACTD, T], F32, tag="psm")
                mm(prep[:], reps[:], en[:])
                prod = wk.tile([NE * ACTD, T], MMDT, tag="prod")
                nc.vector.tensor_tensor(prod[:], mus[:], prep[:], OP.mult)

                # mu_bar = grp.T @ prod + mub.T @ en  [6, T]
                pmu = ps_sm.tile([ACTD, T], F32, tag="psm")
                mm(pmu[:], grps[:], prod[:], start=True, stop=False)
                mm(pmu[:], mubs[:], en[:], start=False, stop=True)
                # s_bar (pre-clip) = ls.T @ en  [6, T]
                psb = ps_sm.tile([ACTD, T], F32, tag="psm")
                mm(psb[:], lss[:], en[:])

                # value head
                pv1 = ps_val.tile([128, 2 * T], F32, tag="pval")
                for m in range(2):
                    mm(pv1[:, m * T:(m + 1) * T], wv1c[0][:, m * 128:(m + 1) * 128], feat[:, 0:T], start=True, stop=False)
                    mm(pv1[:, m * T:(m + 1) * T], wv1c[1][:, m * 128:(m + 1) * 128], feat[:, T:2 * T], start=False, stop=True)
                vh1 = wk.tile([128, 2 * T], MMDT, tag="vh1")
                tanh2(vh1, pv1, (5, 6), bias_zero["bv1"])

                pv2 = ps_val.tile([128, 2 * T], F32, tag="pval")
                for m in range(2):
                    mm(pv2[:, m * T:(m + 1) * T], wv2c[0][:, m * 128:(m + 1) * 128], vh1[:, 0:T], start=True, stop=False)
                    mm(pv2[:, m * T:(m + 1) * T], wv2c[1][:, m * 128:(m + 1) * 128], vh1[:, T:2 * T], start=False, stop=True)
                vh2 = wk.tile([128, 2 * T], MMDT, tag="vh2")
                tanh2(vh2, pv2, (7, 8), bias_zero["bv2"])

                pvv = ps_sm.tile([1, T], F32, tag="psm")
                mm(pvv[:], wv3c[0][:], vh2[:, 0:T], start=True, stop=False)
                mm(pvv[:], wv3c[1][:], vh2[:, T:2 * T], start=False, stop=True)

                # assemble outputs: rows 0-5 mu, 6-11 clipped s, 12 v+bv3
                omu = io.tile([ACTD, T], F32, tag="omu")
                nc.vector.tensor_copy(omu[:], pmu[:])
                nc.sync.dma_start(out13[0:ACTD, cols], omu[:])
                osb = io.tile([ACTD, T], F32, tag="osb")
                nc.vector.tensor_scalar(osb[:], psb[:], LSMIN, LSMAX, OP.max, OP.min)
                nc.sync.dma_start(out13[ACTD:2 * ACTD, cols], osb[:])
                ov = io.tile([1, T], F32, tag="ov")
                if bias_zero["bv3"]:
                    nc.scalar.copy(ov[:], pvv[:])
                else:
                    nc.vector.tensor_scalar(ov[:], pvv[:], bv3t[:], 0.0, OP.add, OP.add)
                nc.sync.dma_start(out13[12:13, cols], ov[:])
    nc.compile()
    return nc


build_nc.bias_zero = {}


def _prep_weights(inputs):
    f32 = np.float32
    g = lambda k: np.ascontiguousarray(np.asarray(inputs[k], dtype=f32))
    W1, b1, W2, b2 = g("W1"), g("b1"), g("W2"), g("b2")
    Wg1, bg1, Wg2, bg2 = g("Wg1"), g("bg1"), g("Wg2"), g("bg2")
    muW, mub, log_std = g("muW"), g("mub"), g("log_std")
    Wv1, bv1, Wv2, bv2, Wv3, bv3 = g("Wv1"), g("bv1"), g("Wv2"), g("bv2"), g("Wv3"), g("bv3")
    centers, mean, var = g("centers"), g("stats_mean"), g("stats_var")

    inv = 1.0 / np.sqrt(var + 1e-6)
    # d2s = relu(fcen.T@feat + brep.T@feat^2 + cbias_tot); see module docstring
    fcen = (-2.0 * S) * (inv * (centers + inv * mean)).T                 # [256,16]
    brep = np.repeat((S * inv * inv)[:, None], NE, axis=1)               # [256,16]
    cbias_tot = (S * np.sum(centers ** 2, axis=1)
                 + 2.0 * S * np.sum(centers * inv * mean, axis=1)
                 + S * np.sum(inv * inv * mean * mean)).astype(f32)
    muwt = np.ascontiguousarray(muW.transpose(1, 0, 2).reshape(HID, NE * ACTD), dtype=f32)

    rep = np.zeros((NE, NE * ACTD), dtype=f32)
    for m in range(NE):
        rep[m, m * ACTD:(m + 1) * ACTD] = 1.0
    grp = np.zeros((NE * ACTD, ACTD), dtype=f32)
    for m in range(NE):
        for a in range(ACTD):
            grp[m * ACTD + a, a] = 1.0

    bias128 = np.stack([b1[:128], b1[128:], b2[:128], b2[128:], bg1,
                        bv1[:128], bv1[128:], bv2[:128], bv2[128:]], axis=1).astype(f32)
    bias16 = np.stack([bg2, cbias_tot], axis=1).astype(f32)

    build_nc.bias_zero = {
        "b1": not np.any(b1), "b2": not np.any(b2), "bg1": not np.any(bg1),
        "bv1": not np.any(bv1), "bv2": not np.any(bv2), "bv3": not np.any(bv3),
    }

    mm_arrays = {
        "w1": W1, "w2": W2, "wg1": Wg1, "wg2": Wg2,
        "wv1": Wv1, "wv2": Wv2, "wv3": Wv3,
        "fcen": fcen, "brep": brep, "muwt": muwt,
        "mub16": mub, "ls16": log_std,
        "rep16": rep, "grp96": grp,
        "ones16_d": np.ones((NE, 1), dtype=f32),
        "ones116_d": np.ones((1, NE), dtype=f32),
    }
    out = {k: to_mm(v) for k, v in mm_arrays.items()}
    out["bias128"] = bias128
    out["bias16"] = bias16
    out["bv3s"] = bv3.reshape(1, 1)
    return out


_RUN_KWARGS = {}  # test.py can inject trace=True etc.


def kernel(**inputs):
    obs = np.asarray(inputs["obs"], dtype=np.float32)
    assert obs.shape == (B, OBS)
    obs_T = np.ascontiguousarray(obs.T)  # [17, B]

    weights = _prep_weights(inputs)
    in_maps = []
    for c in range(NCORES):
        m = dict(weights)
        m["obs_t"] = to_mm(obs_T[:, c * BC:(c + 1) * BC])
        in_maps.append(m)

    nc = build_nc()
    res = bass_utils.run_bass_kernel_spmd(nc, in_maps, core_ids=list(range(NCORES)), **_RUN_KWARGS)
    kernel.last_results = res

    mu = np.empty((B, ACTD), dtype=np.float32)
    sb = np.empty((B, ACTD), dtype=np.float32)
    v = np.empty((B,), dtype=np.float32)
    for c in range(NCORES):
        o = res.results[c]["out13"]
        mu[c * BC:(c + 1) * BC] = o[0:ACTD, :].T
        sb[c * BC:(c + 1) * BC] = o[ACTD:2 * ACTD, :].T
        v[c * BC:(c + 1) * BC] = o[12, :]
    return mu, sb, v


# revision 14
# speedup vs baseline: 1.5799x; 1.5799x over previous
"""Trainium2 Bass kernel for nn_CCMoE (chart-cover MoE policy head).

Strategy: pure data-parallel over 8 NeuronCores (batch sharding). Each core
processes B/8 = 16384 samples in feature-major layout ([feature, batch] tiles,
batch tiled by T=512 columns). All contractions run on the tensor engine,
including partition-dim reductions (ones / replication / group-sum matrices as
stationary operands). Whitening, the locality prior, and all biases are folded
into matmul weights plus activation bias ports; the squared-distance term
||zw||^2 is accumulated directly into the [16,T] distance tile via
column-replicated weights. The masked softmax is computed unnormalized (logits
are bounded, exp-safe); normalization via fast-Newton reciprocal + K=1
broadcast matmul.

Self-contained: shapes/sharding hardcoded for the fixed problem size.
"""
import numpy as np
import ml_dtypes
from contextlib import ExitStack

import concourse.bass as bass
import concourse.bacc as bacc
import concourse.tile as tile
from concourse import mybir
from concourse import bass_utils

# problem dims (hardcoded per contract)
B, OBS, ACTD, NE, HID, GHID = 131072, 17, 6, 16, 256, 128
NCORES = 8
BC = B // NCORES          # 16384 samples per core
T = 512                   # batch-column tile (one fp32 PSUM bank)
NT = BC // T              # 32 tiles per core
R = 24.0
BETA = 1.0
S = BETA / (R * R)
LSMIN, LSMAX = -20.0, 2.0

F32 = mybir.dt.float32
F32R = mybir.dt.float32r
BF16 = mybir.dt.bfloat16
AT = mybir.ActivationFunctionType
OP = mybir.AluOpType

MM_DTYPE = "bf16"         # matmul input dtype: "bf16" | "f32r" | "f32"


def _mmdt():
    return {"bf16": BF16, "f32r": F32R, "f32": F32}[MM_DTYPE]


def to_mm(x):
    """Convert host fp32 array to the matmul input dtype's storage."""
    x = np.ascontiguousarray(x, np.float32)
    if MM_DTYPE == "bf16":
        return np.ascontiguousarray(x.astype(ml_dtypes.bfloat16))
    if MM_DTYPE == "f32r":
        xi = x.view(np.uint32)
        yi = ((xi.astype(np.uint64) + 0x800) & 0xFFFFF000).astype(np.uint32)
        return np.ascontiguousarray(yi.view(np.float32))
    return x


def build_nc():
    MMDT = _mmdt()
    nc = bacc.Bacc("TRN2", target_bir_lowering=False, debug=False)

    dram_in = lambda name, shape: nc.dram_tensor(name, shape, MMDT, kind="ExternalInput").ap()
    obs_t = dram_in("obs_t", [OBS, BC])
    w1 = dram_in("w1", [OBS, HID])
    w2 = dram_in("w2", [HID, HID])
    wg1 = dram_in("wg1", [HID, GHID])
    wg2 = dram_in("wg2", [GHID, NE])
    wv1 = dram_in("wv1", [HID, HID])
    wv2 = dram_in("wv2", [HID, HID])
    wv3 = dram_in("wv3", [HID, 1])
    fcen = dram_in("fcen", [HID, NE])        # -2S*inv*(c + inv*mean), per column i
    brep = dram_in("brep", [HID, NE])        # S*inv^2 replicated across columns
    muwt = dram_in("muwt", [HID, NE * ACTD])
    mub16 = dram_in("mub16", [NE, ACTD])
    ls16 = dram_in("ls16", [NE, ACTD])
    rep16 = dram_in("rep16", [NE, NE * ACTD])
    grp96 = dram_in("grp96", [NE * ACTD, ACTD])
    ones16_d = dram_in("ones16_d", [NE, 1])
    ones116_d = dram_in("ones116_d", [1, NE])
    bias128 = nc.dram_tensor("bias128", [128, 9], F32, kind="ExternalInput").ap()
    bias16 = nc.dram_tensor("bias16", [NE, 2], F32, kind="ExternalInput").ap()
    bv3s = nc.dram_tensor("bv3s", [1, 1], F32, kind="ExternalInput").ap()
    out13 = nc.dram_tensor("out13", [13, BC], F32, kind="ExternalOutput").ap()

    with tile.TileContext(nc) as tc:
        with ExitStack() as ctx:
            cp = ctx.enter_context(tc.tile_pool(name="consts", bufs=1))
            io = ctx.enter_context(tc.tile_pool(name="io", bufs=6))
            wk = ctx.enter_context(tc.tile_pool(name="work", bufs=5))
            wkf = ctx.enter_context(tc.tile_pool(name="workf", bufs=6))
            ps_enc = ctx.enter_context(tc.tile_pool(name="ps_enc", bufs=2, space="PSUM"))
            ps_sm = ctx.enter_context(tc.tile_pool(name="ps_sm", bufs=3, space="PSUM"))
            ps_one = ctx.enter_context(tc.tile_pool(name="ps_one", bufs=1, space="PSUM"))

            # ---- constants into SBUF ----
            def cload(name, src, shape, dt=MMDT):
                t = cp.tile(shape, dt, tag=name)
                nc.sync.dma_start(t[:], src)
                return t

            w1s = cload("w1s", w1[:], [OBS, HID])
            w2c = [cload(f"w2_{k}", w2[k * 128:(k + 1) * 128, :], [128, HID]) for k in range(2)]
            wg1c = [cload(f"wg1_{k}", wg1[k * 128:(k + 1) * 128, :], [128, GHID]) for k in range(2)]
            wg2s = cload("wg2s", wg2[:], [GHID, NE])
            wv1c = [cload(f"wv1_{k}", wv1[k * 128:(k + 1) * 128, :], [128, HID]) for k in range(2)]
            wv2c = [cload(f"wv2_{k}", wv2[k * 128:(k + 1) * 128, :], [128, HID]) for k in range(2)]
            wv3c = [cload(f"wv3_{k}", wv3[k * 128:(k + 1) * 128, :], [128, 1]) for k in range(2)]
            fcenc = [cload(f"fcen_{k}", fcen[k * 128:(k + 1) * 128, :], [128, NE]) for k in range(2)]
            brepc = [cload(f"brep_{k}", brep[k * 128:(k + 1) * 128, :], [128, NE]) for k in range(2)]
            muwtc = [cload(f"muwt_{k}", muwt[k * 128:(k + 1) * 128, :], [128, NE * ACTD]) for k in range(2)]
            mubs = cload("mubs", mub16[:], [NE, ACTD])
            lss = cload("lss", ls16[:], [NE, ACTD])
            reps = cload("reps", rep16[:], [NE, NE * ACTD])
            grps = cload("grps", grp96[:], [NE * ACTD, ACTD])
            ones16 = cload("ones16", ones16_d[:], [NE, 1])
            ones116 = cload("ones116", ones116_d[:], [1, NE])
            b128 = cload("b128", bias128[:], [128, 9], dt=F32)
            b16 = cload("b16", bias16[:], [NE, 2], dt=F32)
            bv3t = cload("bv3t", bv3s[:], [1, 1], dt=F32)

            bg2a = b16[:, 0:1]
            cbias = b16[:, 1:2]

            # bias slots in b128 (columns): 0,1=b1  2,3=b2  4=bg1  5,6=bv1  7,8=bv2
            bias_zero = build_nc.bias_zero  # host-detected zero-bias flags

            def mm(out, lhsT, rhs, start=True, stop=True):
                nc.tensor.matmul(out, lhsT, rhs, start=start, stop=stop)

            def tanh2(out_tile, psum, cols_ab, zero):
                """tanh over a [128, 2T] psum pair; merged when bias is zero."""
                ca, cb = cols_ab
                if zero:
                    nc.scalar.activation(out_tile[:], psum[:], AT.Tanh, bias=0.0)
                else:
                    nc.scalar.activation(out_tile[:, 0:T], psum[:, 0:T], AT.Tanh, bias=b128[:, ca:ca + 1])
                    nc.scalar.activation(out_tile[:, T:2 * T], psum[:, T:2 * T], AT.Tanh, bias=b128[:, cb:cb + 1])

            # ---- main loop: groups of G column-tiles, phase-swept so the PE
            # always has G independent sub-tiles of matmul work in flight ----
            G = 4
            for g in range(NT // G):
                base = g * G
                coll = [slice((base + j) * T, (base + j + 1) * T) for j in range(G)]

                # phase E1: encoder layer 1 for all sub-tiles
                obs_l, h1_l = [], []
                for j in range(G):
                    obs_s = io.tile([OBS, T], MMDT, tag="obs_s")
                    nc.sync.dma_start(obs_s[:], obs_t[:, coll[j]])
                    obs_l.append(obs_s)
                for j in range(G):
                    ph1 = ps_enc.tile([128, 2 * T], F32, tag="penc")
                    mm(ph1[:, 0:T], w1s[:, 0:128], obs_l[j][:])
                    mm(ph1[:, T:2 * T], w1s[:, 128:256], obs_l[j][:])
                    h1 = wk.tile([128, 2 * T], MMDT, tag="h1")
                    tanh2(h1, ph1, (0, 1), bias_zero["b1"])
                    h1_l.append(h1)

                # phase E2: encoder layer 2
                feat_l = []
                for j in range(G):
                    pf = ps_enc.tile([128, 2 * T], F32, tag="penc")
                    for m in range(2):
                        mm(pf[:, m * T:(m + 1) * T], w2c[0][:, m * 128:(m + 1) * 128], h1_l[j][:, 0:T], start=True, stop=False)
                        mm(pf[:, m * T:(m + 1) * T], w2c[1][:, m * 128:(m + 1) * 128], h1_l[j][:, T:2 * T], start=False, stop=True)
                    feat = wkf.tile([128, 2 * T], MMDT, tag="feat")
                    tanh2(feat, pf, (2, 3), bias_zero["b2"])
                    feat_l.append(feat)

                # phase G1: gate hidden + logits + distance tile
                glog_l, d2s_l = [], []
                for j in range(G):
                    feat = feat_l[j]
                    pgh = ps_sm.tile([128, T], F32, tag="psm")
                    mm(pgh[:], wg1c[0][:], feat[:, 0:T], start=True, stop=False)
                    mm(pgh[:], wg1c[1][:], feat[:, T:2 * T], start=False, stop=True)
                    gh = wk.tile([128, T], MMDT, tag="gh")
                    if bias_zero["bg1"]:
                        nc.scalar.activation(gh[:], pgh[:], AT.Tanh, bias=0.0)
                    else:
                        nc.scalar.activation(gh[:], pgh[:], AT.Tanh, bias=b128[:, 4:5])

                    pglog = ps_sm.tile([NE, T], F32, tag="psm")
                    mm(pglog[:], wg2s[:], gh[:])
                    glog_l.append(pglog)

                    f2 = wk.tile([128, 2 * T], MMDT, tag="f2")
                    fview = feat[:].bitcast(F32) if MM_DTYPE == "f32r" else feat[:]
                    nc.vector.tensor_tensor(f2[:], fview, fview, OP.mult)

                    pd2 = ps_sm.tile([NE, T], F32, tag="psm")
                    mm(pd2[:], fcenc[0][:], feat[:, 0:T], start=True, stop=False)
                    mm(pd2[:], fcenc[1][:], feat[:, T:2 * T], start=False, stop=False)
                    mm(pd2[:], brepc[0][:], f2[:, 0:T], start=False, stop=False)
                    mm(pd2[:], brepc[1][:], f2[:, T:2 * T], start=False, stop=True)
                    d2s = wk.tile([NE, T], F32, tag="d2s")
                    nc.vector.tensor_scalar(d2s[:], pd2[:], cbias, 0.0, OP.add, OP.max)
                    d2s_l.append(d2s)

                # phase G2: masked softmax (unnormalized) + normalization
                en_l = []
                for j in range(G):
                    d2s = d2s_l[j]
                    tl = wk.tile([NE, T], F32, tag="tl")
                    nc.vector.scalar_tensor_tensor(tl[:], d2s[:], -1.0, glog_l[j][:], OP.mult, OP.add)
                    eraw = wk.tile([NE, T], F32, tag="eraw")
                    nc.scalar.activation(eraw[:], tl[:], AT.Exp, bias=bg2a)
                    e = wk.tile([NE, T], MMDT, tag="e")
                    nc.vector.scalar_tensor_tensor(e[:], d2s[:], BETA, eraw[:], OP.is_le, OP.mult)

                    pZ = ps_one.tile([1, T], F32, tag="pone")
                    mm(pZ[:], ones16[:], e[:])
                    rz32 = wk.tile([1, T], F32, tag="rz32")
                    nc.vector.reciprocal_approx_fast(rz32[:], pZ[:])
                    rz = wk.tile([1, T], MMDT, tag="rz")
                    nc.vector.tensor_copy(rz[:], rz32[:])
                    prb = ps_sm.tile([NE, T], F32, tag="psm")
                    mm(prb[:], ones116[:], rz[:])
                    en = wk.tile([NE, T], MMDT, tag="en")
                    eview = e[:].bitcast(F32) if MM_DTYPE == "f32r" else e[:]
                    nc.vector.tensor_tensor(en[:], eview, prb[:], OP.mult)
                    en_l.append(en)

                # phase M: expert mixture -> mu, s
                for j in range(G):
                    feat, en = feat_l[j], en_l[j]
                    pmus = ps_sm.tile([NE * ACTD, T], F32, tag="psm")
                    mm(pmus[:], muwtc[0][:], feat[:, 0:T], start=True, stop=False)
                    mm(pmus[:], muwtc[1][:], feat[:, T:2 * T], start=False, stop=True)
                    mus = wk.tile([NE * ACTD, T], F32, tag="mus")
                    nc.vector.tensor_copy(mus[:], pmus[:])
                    prep = ps_sm.tile([NE * ACTD, T], F32, tag="psm")
                    mm(prep[:], reps[:], en[:])
                    prod = wk.tile([NE * ACTD, T], MMDT, tag="prod")
                    nc.vector.tensor_tensor(prod[:], mus[:], prep[:], OP.mult)

                    pmu = ps_sm.tile([ACTD, T], F32, tag="psm")
                    mm(pmu[:], grps[:], prod[:], start=True, stop=False)
                    mm(pmu[:], mubs[:], en[:], start=False, stop=True)
                    psb = ps_sm.tile([ACTD, T], F32, tag="psm")
                    mm(psb[:], lss[:], en[:])

                    omu = io.tile([ACTD, T], F32, tag="omu")
                    nc.vector.tensor_copy(omu[:], pmu[:])
                    nc.sync.dma_start(out13[0:ACTD, coll[j]], omu[:])
                    osb = io.tile([ACTD, T], F32, tag="osb")
                    nc.vector.tensor_scalar(osb[:], psb[:], LSMIN, LSMAX, OP.max, OP.min)
                    nc.sync.dma_start(out13[ACTD:2 * ACTD, coll[j]], osb[:])

                # phase V: value head
                vh2_l = []
                for j in range(G):
                    feat = feat_l[j]
                    pv1 = ps_enc.tile([128, 2 * T], F32, tag="penc")
                    for m in range(2):
                        mm(pv1[:, m * T:(m + 1) * T], wv1c[0][:, m * 128:(m + 1) * 128], feat[:, 0:T], start=True, stop=False)
                        mm(pv1[:, m * T:(m + 1) * T], wv1c[1][:, m * 128:(m + 1) * 128], feat[:, T:2 * T], start=False, stop=True)
                    vh1 = wk.tile([128, 2 * T], MMDT, tag="vh1")
                    tanh2(vh1, pv1, (5, 6), bias_zero["bv1"])

                    pv2 = ps_enc.tile([128, 2 * T], F32, tag="penc")
                    for m in range(2):
                        mm(pv2[:, m * T:(m + 1) * T], wv2c[0][:, m * 128:(m + 1) * 128], vh1[:, 0:T], start=True, stop=False)
                        mm(pv2[:, m * T:(m + 1) * T], wv2c[1][:, m * 128:(m + 1) * 128], vh1[:, T:2 * T], start=False, stop=True)
                    vh2 = wk.tile([128, 2 * T], MMDT, tag="vh2")
                    tanh2(vh2, pv2, (7, 8), bias_zero["bv2"])
                    vh2_l.append(vh2)

                for j in range(G):
                    pvv = ps_one.tile([1, T], F32, tag="pone")
                    mm(pvv[:], wv3c[0][:], vh2_l[j][:, 0:T], start=True, stop=False)
                    mm(pvv[:], wv3c[1][:], vh2_l[j][:, T:2 * T], start=False, stop=True)
                    ov = io.tile([1, T], F32, tag="ov")
                    if bias_zero["bv3"]:
                        nc.scalar.copy(ov[:], pvv[:])
                    else:
                        nc.vector.tensor_scalar(ov[:], pvv[:], bv3t[:], 0.0, OP.add, OP.add)
                    nc.sync.dma_start(out13[12:13, coll[j]], ov[:])
    nc.compile()
    return nc


build_nc.bias_zero = {}


def _prep_weights(inputs):
    f32 = np.float32
    g = lambda k: np.ascontiguousarray(np.asarray(inputs[k], dtype=f32))
    W1, b1, W2, b2 = g("W1"), g("b1"), g("W2"), g("b2")
    Wg1, bg1, Wg2, bg2 = g("Wg1"), g("bg1"), g("Wg2"), g("bg2")
    muW, mub, log_std = g("muW"), g("mub"), g("log_std")
    Wv1, bv1, Wv2, bv2, Wv3, bv3 = g("Wv1"), g("bv1"), g("Wv2"), g("bv2"), g("Wv3"), g("bv3")
    centers, mean, var = g("centers"), g("stats_mean"), g("stats_var")

    inv = 1.0 / np.sqrt(var + 1e-6)
    # d2s = relu(fcen.T@feat + brep.T@feat^2 + cbias_tot); see module docstring
    fcen = (-2.0 * S) * (inv * (centers + inv * mean)).T                 # [256,16]
    brep = np.repeat((S * inv * inv)[:, None], NE, axis=1)               # [256,16]
    cbias_tot = (S * np.sum(centers ** 2, axis=1)
                 + 2.0 * S * np.sum(centers * inv * mean, axis=1)
                 + S * np.sum(inv * inv * mean * mean)).astype(f32)
    muwt = np.ascontiguousarray(muW.transpose(1, 0, 2).reshape(HID, NE * ACTD), dtype=f32)

    rep = np.zeros((NE, NE * ACTD), dtype=f32)
    for m in range(NE):
        rep[m, m * ACTD:(m + 1) * ACTD] = 1.0
    grp = np.zeros((NE * ACTD, ACTD), dtype=f32)
    for m in range(NE):
        for a in range(ACTD):
            grp[m * ACTD + a, a] = 1.0

    bias128 = np.stack([b1[:128], b1[128:], b2[:128], b2[128:], bg1,
                        bv1[:128], bv1[128:], bv2[:128], bv2[128:]], axis=1).astype(f32)
    bias16 = np.stack([bg2, cbias_tot], axis=1).astype(f32)

    build_nc.bias_zero = {
        "b1": not np.any(b1), "b2": not np.any(b2), "bg1": not np.any(bg1),
        "bv1": not np.any(bv1), "bv2": not np.any(bv2), "bv3": not np.any(bv3),
    }

    mm_arrays = {
        "w1": W1, "w2": W2, "wg1": Wg1, "wg2": Wg2,
        "wv1": Wv1, "wv2": Wv2, "wv3": Wv3,
        "fcen": fcen, "brep": brep, "muwt": muwt,
        "mub16": mub, "ls16": log_std,
        "rep16": rep, "grp96": grp,
        "ones16_d": np.ones((NE, 1), dtype=f32),
        "ones116_d": np.ones((1, NE), dtype=f32),
    }
    out = {k: to_mm(v) for k, v in mm_arrays.items()}
    out["bias128"] = bias128
    out["bias16"] = bias16
    out["bv3s"] = bv3.reshape(1, 1)
    return out


_RUN_KWARGS = {}  # test.py can inject trace=True etc.


def kernel(**inputs):
    obs = np.asarray(inputs["obs"], dtype=np.float32)
    assert obs.shape == (B, OBS)
    obs_T = np.ascontiguousarray(obs.T)  # [17, B]

    weights = _prep_weights(inputs)
    in_maps = []
    for c in range(NCORES):
        m = dict(weights)
        m["obs_t"] = to_mm(obs_T[:, c * BC:(c + 1) * BC])
        in_maps.append(m)

    nc = build_nc()
    res = bass_utils.run_bass_kernel_spmd(nc, in_maps, core_ids=list(range(NCORES)), **_RUN_KWARGS)
    kernel.last_results = res

    mu = np.empty((B, ACTD), dtype=np.float32)
    sb = np.empty((B, ACTD), dtype=np.float32)
    v = np.empty((B,), dtype=np.float32)
    for c in range(NCORES):
        o = res.results[c]["out13"]
        mu[c * BC:(c + 1) * BC] = o[0:ACTD, :].T
        sb[c * BC:(c + 1) * BC] = o[ACTD:2 * ACTD, :].T
        v[c * BC:(c + 1) * BC] = o[12, :]
    return mu, sb, v
